# revision 1
# baseline (speedup 1.0000x reference)
"""Trainium2 Bass kernel: sliding-window multi-head attention with ALiBi.

Reference computation (B=2, S=4096, E=512, H=8, D=64, window 513):
    q = (inputs_q @ w_q);  k = (inputs_kv @ w_k);  v = (inputs_kv @ w_v)
    att = softmax(q k^T / 8 + alibi, sliding window +-256)
    out = (att v) @ w_o

Sharding: 8 cores = 2 batches x 4 sequence quarters (1024 q rows per core),
kv slices carry a 256-row zero-padded halo; a host-provided validity column
appended to V makes the softmax denominator (accumulated by the AV matmul)
skip padded rows; the window/ALiBi mask is a multiplicative exp-mask applied
after exp() (G = exp(-slope_h*|rel|) * [|rel|<=256], Toeplitz, shared by all
q blocks).

Scores are computed transposed (S^T[kv, q]) so AV needs no transposes:
lhsT = [V | valid], rhs = P^T gives O^T[d, q] plus the denominator row.

Performance structure (~1.9x over the f32r all-serial version):
  - all matmul operands bf16 (PSUM accumulation stays f32): halves DMA and
    doubles DVE mask-mul throughput (2x_1p); output y stays f32.
  - trapezoid score layout: chunks 0 and 5 of each q block are half dead
    (their valid q cols are column-bounded), so they overlay into one
    256-col PSUM block; scores/exp/mask/AV skip dead halves entirely
    (1280 live cols per iteration instead of 1536).
  - two PSUM waves per iteration (2-bank + 1-bank) with separate exp/mask
    pieces, so wave 1's chain starts while wave 0's scores still run.
  - two-level software pipelining: scores(i+1) are emitted before the
    exp/mask/AV chain of iteration i, and the bcast/norm tail of i-1 is
    emitted after the chain of i, keeping every in-order engine queue free
    of cross-engine round-trip stalls.
  - engine placement: exp on ACT, mask-muls on DVE (GPSIMD tensor_mul
    measured catastrophically slow on HW), reciprocal+normalize on DVE,
    partition-broadcast on GPSIMD, q/k/v PSUM evictions + y staging on ACT;
    per-q-block output projection interleaved into the attention stream.
  - weight DMAs interleaved with rep-0 activation loads so the first
    projection matmul unblocks early.
"""

import os
import sys

if "/opt/trn_rl_repo" not in sys.path:
    sys.path.insert(0, "/opt/trn_rl_repo")

import numpy as np

import concourse.bacc as bacc
import concourse.mybir as mybir
import concourse.tile as tile
from concourse.bass_utils import run_bass_kernel_spmd

# ---------------------------------------------------------------- geometry
B, S, E = 2, 4096, 512
H, D = 8, 64
HD = H * D              # 512
HALF = 256              # window half-width (ATTENTION_WINDOW=512 -> 513 wide)
NCORES = 8
SQ = 4                  # sequence shards per batch
QROWS = S // SQ         # 1024 q rows per core
KVROWS = QROWS + 2 * HALF   # 1536 kv rows per core (with halo)
QB = 4                  # q blocks per core
QBLK = QROWS // QB      # 256 q cols per block
NCH = 6                 # kv chunks per q block
CBLK = 128              # kv chunk rows
SP6 = NCH * QBLK        # 1536: all chunks of a q block side by side

F32 = mybir.dt.float32
BF16 = mybir.dt.bfloat16

# GPSIMD tensor_mul measured catastrophically slow on HW (bf16 elementwise
# on the Q7 DSP); keep all mask-muls on DVE.
BMASK_POOL = os.environ.get("K_BMASK", "dve") == "pool"

GTOT = 1280             # exp'd score cols per iteration

# Score layout: two PSUM waves, each with its own exp/mask piece.
#   wave 0: chunks (c1 c2 c3 [c0|c5])  -> G cols [0:1024)   (2 banks)
#   wave 1: chunk  (c4)                -> G cols [1024:1280) (1 bank)
# Chunks 0 and 5 are half dead (c0's valid q cols always in [0,128),
# c5's in [128,256)) and OVERLAY into one 256-col block, so everything
# in wave 0 is live and one contiguous exp/mask covers it.  Wave 1's
# chain (GPSIMD mask) starts while wave 0's scores are still running.
# chunk -> (wave, col offset in wave)
CH_MAP = {1: (0, 0), 2: (0, 256), 3: (0, 512), 0: (0, 768), 5: (0, 768),
          4: (1, 0)}
CH_GOFF = {1: 0, 2: 256, 3: 512, 0: 768, 5: 768, 4: 1024}
CH_LO = {0: 0, 1: 0, 2: 0, 3: 0, 4: 0, 5: 128}
CH_HI = {0: 128, 1: 256, 2: 256, 3: 256, 4: 256, 5: 256}
# emission order: B wave first (its chain starts earliest), start=True on
# the first chunk of each PSUM bank (zeroes the bank's 2KB region)
CH_EMIT = [(4, True), (1, True), (2, False), (3, True), (0, False),
           (5, False)]
# CH_OFF retained for the host-side G builder (global col offsets)
CH_OFF = CH_GOFF

_CACHE = {}


def _build_program(repeats=1):
    """Build + compile the SPMD program (cached per process)."""
    key = ("nc", repeats)
    if key in _CACHE:
        return _CACHE[key]

    nc = bacc.Bacc("TRN2", target_bir_lowering=False, debug=False,
                   enable_asserts=True)

    xq_d = nc.dram_tensor("xqT", [E, QROWS], BF16, kind="ExternalInput")
    xkv_d = nc.dram_tensor("xkvT", [E, KVROWS], BF16, kind="ExternalInput")
    wq_d = nc.dram_tensor("wq", [E, HD], BF16, kind="ExternalInput")
    wk_d = nc.dram_tensor("wk", [E, HD], BF16, kind="ExternalInput")
    wv_d = nc.dram_tensor("wv", [E, HD], BF16, kind="ExternalInput")
    wo_d = nc.dram_tensor("wo", [HD, E], BF16, kind="ExternalInput")
    g_d = nc.dram_tensor("gmask", [H, 128, GTOT], BF16, kind="ExternalInput")
    val_d = nc.dram_tensor("validc", [128, KVROWS // CBLK], F32,
                           kind="ExternalInput")
    y_d = nc.dram_tensor("y", [QROWS, E], F32, kind="ExternalOutput")

    EXP = mybir.ActivationFunctionType.Exp

    with tile.TileContext(nc) as tc:
        with (
            tc.tile_pool(name="wts", bufs=16) as wts,
            tc.tile_pool(name="bigx", bufs=12) as bigx,
            tc.tile_pool(name="proj", bufs=1) as proj,
            tc.tile_pool(name="pwork", bufs=4) as pwork,
            tc.tile_pool(name="small", bufs=4) as small,
            tc.tile_pool(name="spw0", bufs=2, space="PSUM") as spw0,
            tc.tile_pool(name="spw1", bufs=4, space="PSUM") as spw1,
        ):
            # ---- weight tiles (E on partitions, 4 tiles each); DMAs are
            # issued inside rep 0 interleaved with the activation loads so
            # the first projection matmul unblocks as early as possible.
            def alloc4(name, cols):
                return [wts.tile([128, cols], BF16, tag="w",
                                 name=f"{name}{e}") for e in range(4)]

            def dma4(ts, dram):
                for e in range(4):
                    nc.sync.dma_start(ts[e][:],
                                      dram.ap()[128 * e:128 * (e + 1), :])

            wq_sb = alloc4("wq", HD)
            wk_sb = alloc4("wk", HD)
            wv_sb = alloc4("wv", HD)
            wo_sb = alloc4("wo", E)

            valid_sb = small.tile([128, KVROWS // CBLK], F32, tag="validc",
                                  name="validc")
            ones8 = small.tile([128, H], F32, tag="ones8", name="ones8")
            nc.vector.memset(ones8[:], 1.0)

            # ---- persistent activation tiles
            qt_sb = [proj.tile([128, QROWS], BF16, tag=f"qt{t}", name=f"qt{t}")
                     for t in range(4)]
            kt_sb = [proj.tile([128, KVROWS], BF16, tag=f"kt{t}",
                               name=f"kt{t}") for t in range(4)]
            # V tiles: head h occupies cols [65h, 65h+64), col 65h+64 = valid
            v_sb = [proj.tile([128, 65 * H], BF16, tag=f"v{b}", name=f"v{b}")
                    for b in range(KVROWS // CBLK)]
            ot_sb = [proj.tile([128, QROWS], BF16, tag=f"ot{t}", name=f"ot{t}")
                     for t in range(4)]

            for rep in range(repeats):
                if rep == 0:
                    dma4(wq_sb, wq_d)
                xq_sb = []
                for e in range(4):
                    t = bigx.tile([128, KVROWS], BF16, tag="bigx",
                                  name=f"xq{e}")
                    nc.sync.dma_start(t[:, :QROWS],
                                      xq_d.ap()[128 * e:128 * (e + 1), :])
                    xq_sb.append(t)
                if rep == 0:
                    dma4(wk_sb, wk_d)
                xkv_sb = []
                for e in range(4):
                    t = bigx.tile([128, KVROWS], BF16, tag="bigx",
                                  name=f"xkv{e}")
                    nc.sync.dma_start(t[:],
                                      xkv_d.ap()[128 * e:128 * (e + 1), :])
                    xkv_sb.append(t)
                if rep == 0:
                    dma4(wv_sb, wv_d)
                    nc.sync.dma_start(valid_sb[:], val_d.ap()[:])

                # Toeplitz exp-masks, shared by all q blocks (loaded early so
                # the first mask-mul never waits on DMA)
                g_sb = []
                for h in range(H):
                    t = bigx.tile([128, KVROWS], BF16, tag="bigx",
                                  name=f"g{h}")
                    nc.sync.dma_start(t[:, :GTOT], g_d.ap()[h])
                    g_sb.append(t)
                if rep == 0:
                    dma4(wo_sb, wo_d)

                # ---- projections (descending t: attention starts at h=7,
                # which reads tile 3 first)
                for t in range(3, -1, -1):
                    for n in range(QROWS // 512):
                        ps = spw0.tile([128, 512], F32, tag="spw0",
                                        name="psq")
                        for e in range(4):
                            nc.tensor.matmul(
                                ps[:],
                                lhsT=wq_sb[e][:, 128 * t:128 * (t + 1)],
                                rhs=xq_sb[e][:, 512 * n:512 * (n + 1)],
                                start=(e == 0), stop=(e == 3))
                        nc.vector.tensor_copy(
                            qt_sb[t][:, 512 * n:512 * (n + 1)], ps[:])

                for t in range(3, -1, -1):
                    for n in range(KVROWS // 512):
                        ps = spw0.tile([128, 512], F32, tag="spw0",
                                        name="psk")
                        for e in range(4):
                            nc.tensor.matmul(
                                ps[:],
                                lhsT=wk_sb[e][:, 128 * t:128 * (t + 1)],
                                rhs=xkv_sb[e][:, 512 * n:512 * (n + 1)],
                                start=(e == 0), stop=(e == 3))
                        nc.vector.tensor_copy(
                            kt_sb[t][:, 512 * n:512 * (n + 1)], ps[:])

                for blk in range(KVROWS // CBLK):
                    ps = spw0.tile([128, 512], F32, tag="spw0", name="psv")
                    for e in range(4):
                        nc.tensor.matmul(
                            ps[:],
                            lhsT=xkv_sb[e][:, 128 * blk:128 * (blk + 1)],
                            rhs=wv_sb[e][:],
                            start=(e == 0), stop=(e == 3))
                    vv = v_sb[blk][:].rearrange("p (h c) -> p h c", c=65)
                    nc.scalar.copy(
                        vv[:, :, 0:64],
                        ps[:].rearrange("p (h c) -> p h c", c=64))
                    nc.vector.tensor_scalar_mul(
                        vv[:, :, 64], ones8[:],
                        valid_sb[:, blk:blk + 1])

                # ---- attention, software-pipelined: score matmuls of
                # iteration i+1 are emitted before the exp/mask/AV chain of
                # iteration i so the PE always has independent work queued.
                # Scores are split into two waves (chunks 0-3 / 4-5) with
                # separate PSUM tiles so exp can start before the last score
                # matmul and the chain tail stays short.
                def emit_scores(qb, h, sp):
                    th, ph = h // 2, 64 * (h % 2)
                    qs = qt_sb[th][ph:ph + 64, QBLK * qb:QBLK * (qb + 1)]
                    for c, st in CH_EMIT:
                        k0 = QBLK * qb + CBLK * c
                        w, o = CH_MAP[c]
                        lo, hi = CH_LO[c], CH_HI[c]
                        nc.tensor.matmul(
                            sp[w][:, o + lo:o + hi],
                            lhsT=kt_sb[th][ph:ph + 64, k0:k0 + CBLK],
                            rhs=qs[:, lo:hi],
                            start=st, stop=True,
                            skip_group_check=not st)

                def emit_front(qb, h, sp):
                    # exp/mask per wave: wave 1 (c4) first — its GPSIMD
                    # mask is the longest chain, give it the earliest start
                    pe = [None] * 2
                    pm = [None] * 2
                    for w, cols, gof in ((1, QBLK, 1024), (0, 1024, 0)):
                        pe[w] = pwork.tile([128, cols], BF16, tag=f"pe{w}",
                                           name=f"pe{w}")
                        nc.scalar.activation(pe[w][:], sp[w][:], EXP)
                    for w, cols, gof in ((1, QBLK, 1024), (0, 1024, 0)):
                        pm[w] = pwork.tile([128, cols], BF16, tag=f"pm{w}",
                                           name=f"pm{w}")
                        eng = (nc.gpsimd if (w == 1 and BMASK_POOL)
                               else nc.vector)
                        eng.tensor_mul(pm[w][:], pe[w][:],
                                       g_sb[h][:, gof:gof + cols])
                    ot = spw1.tile([65, QBLK], F32, tag="spw1", name="ot")
                    for i, (c, _) in enumerate(CH_EMIT):
                        w, o = CH_MAP[c]
                        lo, hi = CH_LO[c], CH_HI[c]
                        nc.tensor.matmul(
                            ot[:, lo:hi],
                            lhsT=v_sb[2 * qb + c][:, 65 * h:65 * h + 65],
                            rhs=pm[w][:, o + lo:o + hi],
                            start=(i == 0), stop=(i == len(CH_EMIT) - 1),
                            skip_group_check=(i > 0))
                    rec = small.tile([1, QBLK], F32, tag="rec", name="rec")
                    nc.vector.reciprocal(rec[:], ot[64:65, :])
                    return (qb, h, ot, rec)

                def emit_tail(qb, h, ot, rec):
                    th, ph = h // 2, 64 * (h % 2)
                    bc = pwork.tile([64, QBLK], F32, tag="bc", name="bc")
                    nc.gpsimd.partition_broadcast(bc[:], rec[:])
                    nc.vector.tensor_mul(
                        ot_sb[th][ph:ph + 64, QBLK * qb:QBLK * (qb + 1)],
                        ot[0:64, :], bc[:])
                    if h == 0:
                        emit_yproj(qb)

                def emit_yproj(qb):
                    for yb in (2 * qb, 2 * qb + 1):
                        yp = spw1.tile([128, 512], F32, tag="spw1", name="yp")
                        for t in range(4):
                            nc.tensor.matmul(
                                yp[:],
                                lhsT=ot_sb[t][:, 128 * yb:128 * (yb + 1)],
                                rhs=wo_sb[t][:],
                                start=(t == 0), stop=(t == 3))
                        ys = pwork.tile([128, 512], F32, tag="ys",
                                        name="ystage")
                        nc.scalar.copy(ys[:], yp[:])
                        nc.sync.dma_start(
                            y_d.ap()[128 * yb:128 * (yb + 1), :], ys[:])

                pend_front = None
                pend_tail = None
                for qb in range(QB):
                    for h in range(H - 1, -1, -1):
                        sp = [
                            spw0.tile([128, 1024], F32, tag="spw0",
                                      name="sp0"),
                            spw1.tile([128, QBLK], F32, tag="spw1",
                                      name="sp1"),
                        ]
                        emit_scores(qb, h, sp)
                        done = (emit_front(*pend_front)
                                if pend_front is not None else None)
                        if pend_tail is not None:
                            emit_tail(*pend_tail)
                        pend_tail = done
                        pend_front = (qb, h, sp)
                done = emit_front(*pend_front)
                if pend_tail is not None:
                    emit_tail(*pend_tail)
                emit_tail(*done)

    nc.compile()
    _CACHE[key] = nc
    return nc


def build_in_maps(inputs_q, inputs_kv, w_q, w_k, w_v, w_o):
    """Host-side sharding: slice/transpose/pad per core + mask tensors."""
    np_bf = mybir.dt.np(BF16)
    inputs_q = np.asarray(inputs_q, np.float32)
    inputs_kv = np.asarray(inputs_kv, np.float32)

    wq = np.ascontiguousarray(np.asarray(w_q, np.float32) * 0.125).astype(np_bf)
    wk = np.ascontiguousarray(np.asarray(w_k, np.float32)).astype(np_bf)
    wv = np.ascontiguousarray(np.asarray(w_v, np.float32)).astype(np_bf)
    wo = np.ascontiguousarray(np.asarray(w_o, np.float32)).astype(np_bf)

    # Toeplitz exp-mask, pre-unrolled into the reordered score layout
    # (chunk c at col offset CH_OFF[c]; chunks 0 and 5 overlay one block
    # with disjoint support): rel = i - r - 128c + 256
    slopes = np.array([2.0 ** (-(i + 1)) for i in range(H)], np.float64)
    r = np.arange(128)[:, None]
    i = np.arange(QBLK)[None, :]
    g32 = np.zeros((H, 128, GTOT), np.float32)
    for c in range(NCH):
        rel = i - r - 128 * c + 256
        band = (np.abs(rel) <= HALF)
        off = CH_OFF[c]
        for h in range(H):
            g32[h, :, off:off + QBLK] += (
                np.exp(-slopes[h] * np.abs(rel)) * band).astype(np.float32)
    g = g32.astype(np_bf)

    in_maps = []
    for c in range(NCORES):
        b, sq = divmod(c, SQ)
        g0 = QROWS * sq
        xq = np.ascontiguousarray(
            inputs_q[b, g0:g0 + QROWS, :].T).astype(np_bf)
        kvlo = g0 - HALF
        lo, hi = max(0, kvlo), min(S, g0 + QROWS + HALF)
        xkv = np.zeros((E, KVROWS), np_bf)
        xkv[:, lo - kvlo:hi - kvlo] = inputs_kv[b, lo:hi, :].T.astype(np_bf)
        valid = np.zeros((KVROWS,), np.float32)
        valid[lo - kvlo:hi - kvlo] = 1.0
        validc = np.ascontiguousarray(valid.reshape(KVROWS // CBLK, CBLK).T)
        in_maps.append({
            "xqT": xq, "xkvT": xkv,
            "wq": wq, "wk": wk, "wv": wv, "wo": wo,
            "gmask": g, "validc": validc,
        })
    return in_maps


def assemble_output(results):
    out = np.empty((B, S, E), np.float32)
    for c in range(NCORES):
        b, sq = divmod(c, SQ)
        out[b, QROWS * sq:QROWS * (sq + 1), :] = results[c]["y"]
    return out


def kernel(inputs_q, inputs_kv, w_q, w_k, w_v, w_o):
    nc = _build_program()
    in_maps = build_in_maps(inputs_q, inputs_kv, w_q, w_k, w_v, w_o)
    res = run_bass_kernel_spmd(nc, in_maps, core_ids=list(range(NCORES)))
    return assemble_output(res.results)

